# revision 25
# baseline (speedup 1.0000x reference)
"""GroupedQueryAttention Trainium2 Bass kernel (v3).

Sharding: 8 cores = (B=2) x (G=4 KV groups). Each core computes, for its
(batch b, kv-group g): the 4 query heads' Q/K/V projections, causal flash
attention, and a partial output projection Y^T_g. Host sums the 4 partials
per batch and adds bo.

All device inputs are host-prepacked into their exact SBUF layouts so every
DMA is a contiguous partition-major copy (SWDGE aggregates those into large
packets at ~230GB/s; the two HWDGE queues run ~65GB/s each) and x^T needs
no on-chip transposes.

On-chip layout keeps the token dim on the free axis:
  S^T[s, t]  = (K^T s-block).T @ Q^T        (one matmul per s-block)
  P^T        = exp(scale * S^T + mask)      (ACT, PSUM -> SBUF, bf16)
  O^T[dh, t] += (V s-block).T @ P^T         (PSUM accumulation over s-blocks)
  rowsum     += ones.T @ P^T                (PSUM accumulation, M=1)
  Y^T[dm, t] = sum_c (Wo chunk).T @ O^T_c   (per 128-row dm block)

Schedule: tau0's K/V/Q projections are d-interleaved over 6 PSUM
accumulators so the PE consumes each arriving x/W d-chunk immediately;
later projections and oproj m-blocks are interleaved between attention
pairs as PE filler so the PE never stalls on the exp->PV chain or the
normalize chain (stalls would also drop the PE out of its 2.4GHz p-state).
PV/rowsum consumption runs 2 s-blocks behind exp, crossing pair boundaries.
"""

import sys

sys.path.insert(0, "/opt/trn_rl_repo")

from collections import deque
from contextlib import ExitStack

import ml_dtypes
import numpy as np

import concourse.bass as bass  # noqa: F401
import concourse.tile as tile
from concourse import bacc, mybir
from concourse.bass_utils import run_bass_kernel_spmd

F32 = mybir.dt.float32
BF16 = mybir.dt.bfloat16
AF = mybir.ActivationFunctionType

D = 2048          # model dim
T = 2048          # tokens
DH = 128          # head dim
G = 4             # kv groups
HPG = 4           # query heads per group
QC = HPG * DH     # query cols per group = 512
ND = D // 128     # 16 contraction chunks
NTAU = 4          # t tiles of 512
TW = 512          # t tile width
SW = 2 * DH + QC  # strm0 row elems per d-chunk: wk|wv|wq = 768
SCALE = DH ** -0.5
NEG = -1e30

TRACE = False
TRACE_KW = {}
LAST_RESULTS = None

_CACHE = {}


def _body(ctx, tc, tens):
    nc = tc.nc
    (xtd, strmd, wod, bqd, bkd, bvd, maskd, identd, yb) = tens

    # PSUM: acc(2) + st(3) + ot(2) + rs(1) = 8 banks
    psacc = ctx.enter_context(tc.tile_pool(name="psacc", bufs=2, space="PSUM"))
    psst = ctx.enter_context(tc.tile_pool(name="psst", bufs=3, space="PSUM"))
    psot = ctx.enter_context(tc.tile_pool(name="psot", bufs=2, space="PSUM"))
    psrs = ctx.enter_context(tc.tile_pool(name="psrs", bufs=1, space="PSUM"))

    consts = ctx.enter_context(tc.tile_pool(name="consts", bufs=1))
    ptp = ctx.enter_context(tc.tile_pool(name="ptp", bufs=5))
    vts = ctx.enter_context(tc.tile_pool(name="vstage", bufs=2))
    ysp = ctx.enter_context(tc.tile_pool(name="ybounce", bufs=4))
    rsb = ctx.enter_context(tc.tile_pool(name="rsb", bufs=2))
    rbp = ctx.enter_context(tc.tile_pool(name="rcb", bufs=2))

    # persistent SBUF tiles
    xts = consts.tile([128, NTAU, ND, TW], BF16, tag="xts")
    strm = consts.tile([128, ND, SW], BF16, tag="strm")   # wk|wv|wq per d
    wot = consts.tile([128, HPG, D], BF16, tag="wot")
    bqt = consts.tile([128, HPG], F32, tag="bqt")
    bkt = consts.tile([128, 1], F32, tag="bkt")
    bvt = consts.tile([128, 1], F32, tag="bvt")
    maskt = consts.tile([128, 128], F32, tag="maskt")
    ident = consts.tile([128, 128], BF16, tag="ident")
    kt = consts.tile([128, T], BF16, tag="kt")
    qts = consts.tile([128, HPG, T], BF16, tag="qts")
    vv = consts.tile([128, ND, DH], BF16, tag="vv")
    ots = consts.tile([128, HPG, T], BF16, tag="ots")
    ones128 = consts.tile([128, 128], BF16, tag="ones128")
    rst = psrs.tile([128, TW], F32, tag="rs")

    def wk_d(d):
        return strm[:, d, 0:DH]

    def wv_d(d):
        return strm[:, d, DH:2 * DH]

    def wq_d(d, cb):
        return strm[:, d, 2 * DH + cb * 128:2 * DH + (cb + 1) * 128]

    # ---- DMA schedule ----
    # gpsimd SWDGE (bulk, aggregating): strm/x-tg0 interleaved per d-pair
    # in consumption order, then x tg1, tg2
    for d0, d1 in ((0, 1), (1, 2), (2, 4), (4, 6), (6, 8), (8, 10), (10, 12)):
        nc.gpsimd.dma_start(strm[:, d0:d1, :], strmd[:, d0:d1, :])
        nc.gpsimd.dma_start(xts[:, 0, d0:d1, :], xtd[:, 0, d0:d1, :])
    nc.gpsimd.dma_start(strm[:, 12:16, :], strmd[:, 12:16, :])
    for p in range(4):
        nc.gpsimd.dma_start(xts[:, 1, 4 * p:4 * p + 4, :],
                            xtd[:, 1, 4 * p:4 * p + 4, :])
    nc.gpsimd.dma_start(xts[:, 2], xtd[:, 2])
    # sync HWDGE: x tg0 d12-15 (needed ~30us), wo m0-7, x tg3 first half
    nc.sync.dma_start(xts[:, 0, 12:16, :], xtd[:, 0, 12:16, :])
    nc.sync.dma_start(wot[:, :, 0:1024], wod[:, :, 0:1024])
    nc.sync.dma_start(xts[:, 3, 0:8, :], xtd[:, 3, 0:8, :])
    # scalar HWDGE: small consts, wo m8-15, x tg3 second half
    nc.scalar.dma_start(bkt, bkd)
    nc.scalar.dma_start(bvt, bvd)
    nc.scalar.dma_start(bqt, bqd)
    nc.scalar.dma_start(ident, identd)
    nc.scalar.dma_start(maskt, maskd)
    nc.scalar.dma_start(wot[:, :, 1024:2048], wod[:, :, 1024:2048])
    nc.scalar.dma_start(xts[:, 3, 8:16, :], xtd[:, 3, 8:16, :])
    nc.vector.memset(ones128, 1.0)

    # ---- per-tau projection block: K/V/Q0-2 d-interleaved over 5
    # accumulators (consumes x/W chunks as they arrive), then Q3 ----
    def emit_block(tau):
        accK = psacc.tile([128, TW], F32, tag="acc", name="accK")
        accV = psacc.tile([128, TW], F32, tag="acc", name="accV")
        accQ = [psst.tile([128, TW], F32, tag="st", name=f"accQ{i}")
                for i in range(3)]
        off = tau * TW
        for d in range(ND):
            xd = xts[:, tau, d, :]
            nc.tensor.matmul(accK, wk_d(d), xd, start=(d == 0),
                             stop=(d == ND - 1))
            nc.tensor.matmul(accV, wv_d(d), xd, start=(d == 0),
                             stop=(d == ND - 1))
            for cb in range(3):
                nc.tensor.matmul(accQ[cb], wq_d(d, cb), xd, start=(d == 0),
                                 stop=(d == ND - 1))
        nc.vector.tensor_scalar_add(kt[:, off:off + TW], accK, bkt[:, 0:1])
        for cb in range(3):
            nc.vector.tensor_scalar_add(qts[:, cb, off:off + TW], accQ[cb],
                                        bqt[:, cb:cb + 1])
        accQ3 = psacc.tile([128, TW], F32, tag="acc", name="accQ3")
        for d in range(ND):
            nc.tensor.matmul(accQ3, wq_d(d, 3), xts[:, tau, d, :],
                             start=(d == 0), stop=(d == ND - 1))
        vtt = vts.tile([128, TW], BF16, tag="vt")
        nc.vector.tensor_scalar_add(vtt, accV, bvt[:, 0:1])
        for i in range(4):
            pv = psst.tile([128, 128], BF16, tag="st", name="pv")
            nc.tensor.transpose(pv, vtt[:, i * 128:(i + 1) * 128], ident)
            nc.vector.tensor_copy(vv[:, tau * 4 + i, :], pv)
        nc.vector.tensor_scalar_add(qts[:, 3, off:off + TW], accQ3,
                                    bqt[:, 3:4])

    # ---- output-projection block emitter ----
    def emit_oproj_block(tau, m):
        yp = psacc.tile([128, TW], F32, tag="acc", name="yp")
        for c in range(HPG):
            nc.tensor.matmul(yp, wot[:, c, m * 128:(m + 1) * 128],
                             ots[:, c, tau * TW:(tau + 1) * TW],
                             start=(c == 0), stop=(c == HPG - 1))
        ys = ysp.tile([128, TW], BF16, tag="ys")
        tail = tau == 3 or (tau == 2 and m >= 12)
        eng = nc.scalar if (tail and m % 2 == 1) else nc.vector
        if eng is nc.scalar:
            nc.scalar.copy(ys, yp)
        else:
            nc.vector.tensor_copy(ys, yp)
        dma_eng = nc.scalar if (tail and m % 2 == 1) else nc.sync
        dma_eng.dma_start(yb[m, tau], ys)

    # ---- attention ----
    # pending: PV + rowsum matmuls lag exp by 3 s-blocks. The rowsum uses a
    # ones [128,128] stationary so its PE tile config matches every other
    # matmul (an M=1 output costs a ~100ns array-reconfig on entry and exit);
    # all 128 result rows hold the rowsum, which doubles as the partition
    # broadcast the normalize needs.
    pending = deque()

    def consume_one():
        otp, pt, lo, sb, first, last, tau, j = pending.popleft()
        nc.tensor.matmul(otp[:, lo:], vv[:, sb, :], pt[:, lo:],
                         start=first, stop=last)
        nc.tensor.matmul(rst[:, lo:], ones128, pt[:, lo:],
                         start=first, stop=last)
        if last:
            # normalize chain, off the PE stream
            rss = rsb.tile([128, TW], F32, tag="rss")
            nc.vector.tensor_copy(rss, rst)
            rcb = rbp.tile([128, TW], F32, tag="rcb")
            nc.vector.reciprocal_approx_fast(rcb, rss)
            nc.vector.tensor_mul(ots[:, j, tau * TW:(tau + 1) * TW], otp, rcb)

    def emit_pair(tau, j):
        nsb = 4 * tau + 4
        otp = psot.tile([128, TW], F32, tag="ot")
        qoff = tau * TW
        for sb in range(nsb):
            di = sb - 4 * tau
            lo = di * 128 if di >= 0 else 0
            st = psst.tile([128, TW], F32, tag="st")
            nc.tensor.matmul(st[:, lo:], kt[:, sb * 128:(sb + 1) * 128],
                             qts[:, j, qoff + lo:qoff + TW],
                             start=True, stop=True)
            if di >= 0:
                nc.vector.tensor_add(st[:, lo:lo + 128], st[:, lo:lo + 128],
                                     maskt)
            pt = ptp.tile([128, TW], BF16, tag="pt")
            nc.scalar.activation(pt[:, lo:], st[:, lo:], AF.Exp, scale=SCALE)
            pending.append((otp, pt, lo, sb, sb == 0, sb == nsb - 1, tau, j))
            while len(pending) > 4:
                consume_one()

    # ---- main schedule ----
    emit_block(0)
    for tau in range(NTAU):
        for j in range(HPG):
            emit_pair(tau, j)
            if tau >= 1 and not (tau == 3 and j == 3):
                for m in range(4 * j, 4 * j + 4):
                    emit_oproj_block(tau - 1, m)
        if tau < 3:
            emit_block(tau + 1)

    # tail: remaining oproj(2) blocks interleaved with the pending flush
    emit_oproj_block(2, 12)
    if pending:
        consume_one()
    emit_oproj_block(2, 13)
    if pending:
        consume_one()
    emit_oproj_block(2, 14)
    while pending:
        consume_one()
    emit_oproj_block(2, 15)
    for m in range(ND):
        emit_oproj_block(3, m)


def _build_nc():
    if "nc" in _CACHE:
        return _CACHE["nc"]
    nc = bacc.Bacc("TRN2", target_bir_lowering=False, debug=False)
    xtd = nc.dram_tensor("xt", [128, NTAU, ND, TW], BF16,
                         kind="ExternalInput").ap()
    strmd = nc.dram_tensor("strm", [128, ND, SW], BF16,
                           kind="ExternalInput").ap()
    wod = nc.dram_tensor("wo", [128, HPG, D], BF16, kind="ExternalInput").ap()
    bqd = nc.dram_tensor("bq", [128, HPG], F32, kind="ExternalInput").ap()
    bkd = nc.dram_tensor("bk", [128, 1], F32, kind="ExternalInput").ap()
    bvd = nc.dram_tensor("bv", [128, 1], F32, kind="ExternalInput").ap()
    maskd = nc.dram_tensor("mask", [128, 128], F32, kind="ExternalInput").ap()
    identd = nc.dram_tensor("ident", [128, 128], BF16,
                            kind="ExternalInput").ap()
    yb = nc.dram_tensor("yb", [ND, NTAU, 128, TW], BF16,
                        kind="ExternalOutput").ap()

    tens = (xtd, strmd, wod, bqd, bkd, bvd, maskd, identd, yb)
    with tile.TileContext(nc) as tc, ExitStack() as ctx:
        _body(ctx, tc, tens)
    nc.compile()
    _CACHE["nc"] = nc
    return nc


def _host_consts():
    p = np.arange(128)[:, None]
    f = np.arange(128)[None, :]
    masks = np.where(f >= p, 0.0, NEG).astype(np.float32)
    ident = np.eye(128, dtype=ml_dtypes.bfloat16)
    return masks, ident


def kernel(x, Wq, bq, Wk, bk, Wv, bv, Wo, bo):
    global LAST_RESULTS
    x = np.asarray(x, np.float32)
    Wq = np.asarray(Wq, np.float32)
    Wk = np.asarray(Wk, np.float32)
    Wv = np.asarray(Wv, np.float32)
    Wo = np.asarray(Wo, np.float32)
    bq = np.asarray(bq, np.float32)
    bk = np.asarray(bk, np.float32)
    bv = np.asarray(bv, np.float32)
    bo = np.asarray(bo, np.float32)

    nc = _build_nc()
    masks, ident = _host_consts()
    bf = lambda a: np.ascontiguousarray(a).astype(ml_dtypes.bfloat16)

    in_maps = []
    for c in range(8):
        b, g = divmod(c, G)
        xt = x[b].T.reshape(ND, 128, NTAU, TW).transpose(1, 2, 0, 3)
        wk = Wk[:, g * DH:(g + 1) * DH].reshape(ND, 128, DH).transpose(1, 0, 2)
        wv = Wv[:, g * DH:(g + 1) * DH].reshape(ND, 128, DH).transpose(1, 0, 2)
        wq = Wq[:, g * QC:(g + 1) * QC].reshape(ND, 128, QC).transpose(1, 0, 2)
        strm = np.concatenate([wk, wv, wq], axis=2)
        wo = Wo[g * QC:(g + 1) * QC, :].reshape(HPG, 128, D).transpose(1, 0, 2)
        in_maps.append({
            "xt": bf(xt),
            "strm": bf(strm),
            "wo": bf(wo),
            "bq": np.ascontiguousarray(
                bq[g * QC:(g + 1) * QC].reshape(HPG, 128).T),
            "bk": np.ascontiguousarray(
                bk[g * DH:(g + 1) * DH].reshape(128, 1)),
            "bv": np.ascontiguousarray(
                bv[g * DH:(g + 1) * DH].reshape(128, 1)),
            "mask": masks,
            "ident": ident,
        })

    res = run_bass_kernel_spmd(nc, in_maps, list(range(8)), trace=TRACE,
                               **TRACE_KW)
    LAST_RESULTS = res

    y = np.empty((2, T, D), np.float32)
    for b in range(2):
        acc = np.zeros((D, T), np.float32)
        for g in range(G):
            blk = res.results[b * G + g]["yb"].astype(np.float32)
            acc += blk.transpose(0, 2, 1, 3).reshape(D, T)
        y[b] = acc.T + bo
    return y


# revision 26
# speedup vs baseline: 1.1804x; 1.1804x over previous
"""GroupedQueryAttention Trainium2 Bass kernel (v3).

Sharding: 8 cores = (B=2) x (G=4 KV groups). Each core computes, for its
(batch b, kv-group g): the 4 query heads' Q/K/V projections, causal flash
attention, and a partial output projection Y^T_g. Host sums the 4 partials
per batch and adds bo.

All device inputs are host-prepacked into their exact SBUF layouts so every
DMA is a contiguous partition-major copy (SWDGE aggregates those into large
packets at ~230GB/s; the two HWDGE queues run ~65GB/s each) and x^T needs
no on-chip transposes.

On-chip layout keeps the token dim on the free axis:
  S^T[s, t]  = (K^T s-block).T @ Q^T        (one matmul per s-block)
  P^T        = exp(scale * S^T + mask)      (ACT, PSUM -> SBUF, bf16)
  O^T[dh, t] += (V s-block).T @ P^T         (PSUM accumulation over s-blocks)
  rowsum     += ones.T @ P^T                (PSUM accumulation, M=1)
  Y^T[dm, t] = sum_c (Wo chunk).T @ O^T_c   (per 128-row dm block)

Schedule: tau0's K/V/Q projections are d-interleaved over 6 PSUM
accumulators so the PE consumes each arriving x/W d-chunk immediately;
later projections and oproj m-blocks are interleaved between attention
pairs as PE filler so the PE never stalls on the exp->PV chain or the
normalize chain (stalls would also drop the PE out of its 2.4GHz p-state).
PV/rowsum consumption runs 2 s-blocks behind exp, crossing pair boundaries.
"""

import sys

sys.path.insert(0, "/opt/trn_rl_repo")

from collections import deque
from contextlib import ExitStack

import ml_dtypes
import numpy as np

import concourse.bass as bass  # noqa: F401
import concourse.tile as tile
from concourse import bacc, mybir
from concourse.bass_utils import run_bass_kernel_spmd

F32 = mybir.dt.float32
BF16 = mybir.dt.bfloat16
AF = mybir.ActivationFunctionType

D = 2048          # model dim
T = 2048          # tokens
DH = 128          # head dim
G = 4             # kv groups
HPG = 4           # query heads per group
QC = HPG * DH     # query cols per group = 512
ND = D // 128     # 16 contraction chunks
NTAU = 4          # t tiles of 512
TW = 512          # t tile width
SW = 2 * DH + QC  # strm0 row elems per d-chunk: wk|wv|wq = 768
SCALE = DH ** -0.5
NEG = -1e30

TRACE = False
TRACE_KW = {}
LAST_RESULTS = None

_CACHE = {}


def _body(ctx, tc, tens):
    nc = tc.nc
    (xtd, strmd, wod, bqd, bkd, bvd, maskd, identd, yb) = tens

    # PSUM: acc(2) + st(3) + ot(2) + rs(1) = 8 banks
    psacc = ctx.enter_context(tc.tile_pool(name="psacc", bufs=2, space="PSUM"))
    psst = ctx.enter_context(tc.tile_pool(name="psst", bufs=3, space="PSUM"))
    psot = ctx.enter_context(tc.tile_pool(name="psot", bufs=2, space="PSUM"))
    psrs = ctx.enter_context(tc.tile_pool(name="psrs", bufs=1, space="PSUM"))

    consts = ctx.enter_context(tc.tile_pool(name="consts", bufs=1))
    ptp = ctx.enter_context(tc.tile_pool(name="ptp", bufs=5))
    vts = ctx.enter_context(tc.tile_pool(name="vstage", bufs=2))
    ysp = ctx.enter_context(tc.tile_pool(name="ybounce", bufs=4))
    rsb = ctx.enter_context(tc.tile_pool(name="rsb", bufs=2))
    rbp = ctx.enter_context(tc.tile_pool(name="rcb", bufs=2))

    # persistent SBUF tiles
    xts = consts.tile([128, NTAU, ND, TW], BF16, tag="xts")
    strm = consts.tile([128, ND, SW], BF16, tag="strm")   # wk|wv|wq per d
    wot = consts.tile([128, HPG, D], BF16, tag="wot")
    bqt = consts.tile([128, HPG], F32, tag="bqt")
    bkt = consts.tile([128, 1], F32, tag="bkt")
    bvt = consts.tile([128, 1], F32, tag="bvt")
    maskt = consts.tile([128, 128], F32, tag="maskt")
    ident = consts.tile([128, 128], BF16, tag="ident")
    kt = consts.tile([128, T], BF16, tag="kt")
    qts = consts.tile([128, HPG, T], BF16, tag="qts")
    vv = consts.tile([128, ND, DH], BF16, tag="vv")
    ots = consts.tile([128, HPG, T], BF16, tag="ots")
    ones128 = consts.tile([128, 128], BF16, tag="ones128")
    rst = psrs.tile([128, TW], F32, tag="rs")

    def wk_d(d):
        return strm[:, d, 0:DH]

    def wv_d(d):
        return strm[:, d, DH:2 * DH]

    def wq_d(d, cb):
        return strm[:, d, 2 * DH + cb * 128:2 * DH + (cb + 1) * 128]

    # ---- DMA schedule ----
    # gpsimd SWDGE (bulk, aggregating): strm/x-tg0 interleaved per d-pair
    # in consumption order, then x tg1, tg2
    for p in range(8):
        nc.gpsimd.dma_start(strm[:, 2 * p:2 * p + 2, :],
                            strmd[:, 2 * p:2 * p + 2, :])
        nc.gpsimd.dma_start(xts[:, 0, 2 * p:2 * p + 2, :],
                            xtd[:, 0, 2 * p:2 * p + 2, :])
    for p in range(4):
        nc.gpsimd.dma_start(xts[:, 1, 4 * p:4 * p + 4, :],
                            xtd[:, 1, 4 * p:4 * p + 4, :])
    nc.gpsimd.dma_start(xts[:, 2], xtd[:, 2])
    # sync HWDGE: wo m0-7, x tg3 first half
    nc.sync.dma_start(wot[:, :, 0:1024], wod[:, :, 0:1024])
    nc.sync.dma_start(xts[:, 3, 0:8, :], xtd[:, 3, 0:8, :])
    # scalar HWDGE: small consts, wo m8-15, x tg3 second half
    nc.scalar.dma_start(bkt, bkd)
    nc.scalar.dma_start(bvt, bvd)
    nc.scalar.dma_start(bqt, bqd)
    nc.scalar.dma_start(ident, identd)
    nc.scalar.dma_start(maskt, maskd)
    nc.scalar.dma_start(wot[:, :, 1024:2048], wod[:, :, 1024:2048])
    nc.scalar.dma_start(xts[:, 3, 8:16, :], xtd[:, 3, 8:16, :])
    nc.vector.memset(ones128, 1.0)

    # ---- per-tau projection block: K/V/Q0-2 d-interleaved over 5
    # accumulators (consumes x/W chunks as they arrive), then Q3 ----
    def emit_block(tau):
        accK = psacc.tile([128, TW], F32, tag="acc", name="accK")
        accV = psacc.tile([128, TW], F32, tag="acc", name="accV")
        accQ = [psst.tile([128, TW], F32, tag="st", name=f"accQ{i}")
                for i in range(3)]
        off = tau * TW
        for d in range(ND):
            xd = xts[:, tau, d, :]
            nc.tensor.matmul(accK, wk_d(d), xd, start=(d == 0),
                             stop=(d == ND - 1))
            nc.tensor.matmul(accV, wv_d(d), xd, start=(d == 0),
                             stop=(d == ND - 1))
            for cb in range(3):
                nc.tensor.matmul(accQ[cb], wq_d(d, cb), xd, start=(d == 0),
                                 stop=(d == ND - 1))
        nc.vector.tensor_scalar_add(kt[:, off:off + TW], accK, bkt[:, 0:1])
        for cb in range(3):
            nc.vector.tensor_scalar_add(qts[:, cb, off:off + TW], accQ[cb],
                                        bqt[:, cb:cb + 1])
        accQ3 = psacc.tile([128, TW], F32, tag="acc", name="accQ3")
        for d in range(ND):
            nc.tensor.matmul(accQ3, wq_d(d, 3), xts[:, tau, d, :],
                             start=(d == 0), stop=(d == ND - 1))
        vtt = vts.tile([128, TW], BF16, tag="vt")
        nc.vector.tensor_scalar_add(vtt, accV, bvt[:, 0:1])
        for i in range(4):
            pv = psst.tile([128, 128], BF16, tag="st", name="pv")
            nc.tensor.transpose(pv, vtt[:, i * 128:(i + 1) * 128], ident)
            nc.vector.tensor_copy(vv[:, tau * 4 + i, :], pv)
        nc.vector.tensor_scalar_add(qts[:, 3, off:off + TW], accQ3,
                                    bqt[:, 3:4])

    # ---- output-projection block emitter ----
    def emit_oproj_block(tau, m):
        yp = psacc.tile([128, TW], F32, tag="acc", name="yp")
        for c in range(HPG):
            nc.tensor.matmul(yp, wot[:, c, m * 128:(m + 1) * 128],
                             ots[:, c, tau * TW:(tau + 1) * TW],
                             start=(c == 0), stop=(c == HPG - 1))
        ys = ysp.tile([128, TW], BF16, tag="ys")
        tail = tau == 3 or (tau == 2 and m >= 12)
        eng = nc.scalar if (tail and m % 2 == 1) else nc.vector
        if eng is nc.scalar:
            nc.scalar.copy(ys, yp)
        else:
            nc.vector.tensor_copy(ys, yp)
        dma_eng = nc.scalar if (tail and m % 2 == 1) else nc.sync
        dma_eng.dma_start(yb[m, tau], ys)

    # ---- attention ----
    # pending: PV + rowsum matmuls lag exp by 3 s-blocks. The rowsum uses a
    # ones [128,128] stationary so its PE tile config matches every other
    # matmul (an M=1 output costs a ~100ns array-reconfig on entry and exit);
    # all 128 result rows hold the rowsum, which doubles as the partition
    # broadcast the normalize needs.
    pending = deque()

    def consume_one():
        otp, pt, lo, sb, first, last, tau, j = pending.popleft()
        nc.tensor.matmul(otp[:, lo:], vv[:, sb, :], pt[:, lo:],
                         start=first, stop=last)
        nc.tensor.matmul(rst[:, lo:], ones128, pt[:, lo:],
                         start=first, stop=last)
        if last:
            # normalize chain, off the PE stream
            rss = rsb.tile([128, TW], F32, tag="rss")
            nc.vector.tensor_copy(rss, rst)
            rcb = rbp.tile([128, TW], F32, tag="rcb")
            nc.vector.reciprocal_approx_fast(rcb, rss)
            nc.vector.tensor_mul(ots[:, j, tau * TW:(tau + 1) * TW], otp, rcb)

    def emit_pair(tau, j):
        nsb = 4 * tau + 4
        otp = psot.tile([128, TW], F32, tag="ot")
        qoff = tau * TW
        for sb in range(nsb):
            di = sb - 4 * tau
            lo = di * 128 if di >= 0 else 0
            st = psst.tile([128, TW], F32, tag="st")
            nc.tensor.matmul(st[:, lo:], kt[:, sb * 128:(sb + 1) * 128],
                             qts[:, j, qoff + lo:qoff + TW],
                             start=True, stop=True)
            if di >= 0:
                nc.vector.tensor_add(st[:, lo:lo + 128], st[:, lo:lo + 128],
                                     maskt)
            pt = ptp.tile([128, TW], BF16, tag="pt")
            nc.scalar.activation(pt[:, lo:], st[:, lo:], AF.Exp, scale=SCALE)
            pending.append((otp, pt, lo, sb, sb == 0, sb == nsb - 1, tau, j))
            while len(pending) > 4:
                consume_one()

    # ---- main schedule ----
    emit_block(0)
    for tau in range(NTAU):
        for j in range(HPG):
            emit_pair(tau, j)
            if tau >= 1 and not (tau == 3 and j == 3):
                for m in range(4 * j, 4 * j + 4):
                    emit_oproj_block(tau - 1, m)
        if tau < 3:
            emit_block(tau + 1)

    # tail: remaining oproj(2) blocks interleaved with the pending flush
    emit_oproj_block(2, 12)
    if pending:
        consume_one()
    emit_oproj_block(2, 13)
    if pending:
        consume_one()
    emit_oproj_block(2, 14)
    while pending:
        consume_one()
    emit_oproj_block(2, 15)
    for m in range(ND):
        emit_oproj_block(3, m)


def _build_nc():
    if "nc" in _CACHE:
        return _CACHE["nc"]
    nc = bacc.Bacc("TRN2", target_bir_lowering=False, debug=False)
    xtd = nc.dram_tensor("xt", [128, NTAU, ND, TW], BF16,
                         kind="ExternalInput").ap()
    strmd = nc.dram_tensor("strm", [128, ND, SW], BF16,
                           kind="ExternalInput").ap()
    wod = nc.dram_tensor("wo", [128, HPG, D], BF16, kind="ExternalInput").ap()
    bqd = nc.dram_tensor("bq", [128, HPG], F32, kind="ExternalInput").ap()
    bkd = nc.dram_tensor("bk", [128, 1], F32, kind="ExternalInput").ap()
    bvd = nc.dram_tensor("bv", [128, 1], F32, kind="ExternalInput").ap()
    maskd = nc.dram_tensor("mask", [128, 128], F32, kind="ExternalInput").ap()
    identd = nc.dram_tensor("ident", [128, 128], BF16,
                            kind="ExternalInput").ap()
    yb = nc.dram_tensor("yb", [ND, NTAU, 128, TW], BF16,
                        kind="ExternalOutput").ap()

    tens = (xtd, strmd, wod, bqd, bkd, bvd, maskd, identd, yb)
    with tile.TileContext(nc) as tc, ExitStack() as ctx:
        _body(ctx, tc, tens)
    nc.compile()
    _CACHE["nc"] = nc
    return nc


def _host_consts():
    p = np.arange(128)[:, None]
    f = np.arange(128)[None, :]
    masks = np.where(f >= p, 0.0, NEG).astype(np.float32)
    ident = np.eye(128, dtype=ml_dtypes.bfloat16)
    return masks, ident


def kernel(x, Wq, bq, Wk, bk, Wv, bv, Wo, bo):
    global LAST_RESULTS
    x = np.asarray(x, np.float32)
    Wq = np.asarray(Wq, np.float32)
    Wk = np.asarray(Wk, np.float32)
    Wv = np.asarray(Wv, np.float32)
    Wo = np.asarray(Wo, np.float32)
    bq = np.asarray(bq, np.float32)
    bk = np.asarray(bk, np.float32)
    bv = np.asarray(bv, np.float32)
    bo = np.asarray(bo, np.float32)

    nc = _build_nc()
    masks, ident = _host_consts()
    bf = lambda a: np.ascontiguousarray(a).astype(ml_dtypes.bfloat16)

    in_maps = []
    for c in range(8):
        b, g = divmod(c, G)
        xt = x[b].T.reshape(ND, 128, NTAU, TW).transpose(1, 2, 0, 3)
        wk = Wk[:, g * DH:(g + 1) * DH].reshape(ND, 128, DH).transpose(1, 0, 2)
        wv = Wv[:, g * DH:(g + 1) * DH].reshape(ND, 128, DH).transpose(1, 0, 2)
        wq = Wq[:, g * QC:(g + 1) * QC].reshape(ND, 128, QC).transpose(1, 0, 2)
        strm = np.concatenate([wk, wv, wq], axis=2)
        wo = Wo[g * QC:(g + 1) * QC, :].reshape(HPG, 128, D).transpose(1, 0, 2)
        in_maps.append({
            "xt": bf(xt),
            "strm": bf(strm),
            "wo": bf(wo),
            "bq": np.ascontiguousarray(
                bq[g * QC:(g + 1) * QC].reshape(HPG, 128).T),
            "bk": np.ascontiguousarray(
                bk[g * DH:(g + 1) * DH].reshape(128, 1)),
            "bv": np.ascontiguousarray(
                bv[g * DH:(g + 1) * DH].reshape(128, 1)),
            "mask": masks,
            "ident": ident,
        })

    res = run_bass_kernel_spmd(nc, in_maps, list(range(8)), trace=TRACE,
                               **TRACE_KW)
    LAST_RESULTS = res

    y = np.empty((2, T, D), np.float32)
    for b in range(2):
        acc = np.zeros((D, T), np.float32)
        for g in range(G):
            blk = res.results[b * G + g]["yb"].astype(np.float32)
            acc += blk.transpose(0, 2, 1, 3).reshape(D, T)
        y[b] = acc.T + bo
    return y


# revision 27
# speedup vs baseline: 1.1901x; 1.0083x over previous
"""GroupedQueryAttention Trainium2 Bass kernel (v3).

Sharding: 8 cores = (B=2) x (G=4 KV groups). Each core computes, for its
(batch b, kv-group g): the 4 query heads' Q/K/V projections, causal flash
attention, and a partial output projection Y^T_g. Host sums the 4 partials
per batch and adds bo.

All device inputs are host-prepacked into their exact SBUF layouts so every
DMA is a contiguous partition-major copy (SWDGE aggregates those into large
packets at ~230GB/s; the two HWDGE queues run ~65GB/s each) and x^T needs
no on-chip transposes.

On-chip layout keeps the token dim on the free axis:
  S^T[s, t]  = (K^T s-block).T @ Q^T        (one matmul per s-block)
  P^T        = exp(scale * S^T + mask)      (ACT, PSUM -> SBUF, bf16)
  O^T[dh, t] += (V s-block).T @ P^T         (PSUM accumulation over s-blocks)
  rowsum     += ones.T @ P^T                (PSUM accumulation, M=1)
  Y^T[dm, t] = sum_c (Wo chunk).T @ O^T_c   (per 128-row dm block)

Schedule: tau0's K/V/Q projections are d-interleaved over 6 PSUM
accumulators so the PE consumes each arriving x/W d-chunk immediately;
later projections and oproj m-blocks are interleaved between attention
pairs as PE filler so the PE never stalls on the exp->PV chain or the
normalize chain (stalls would also drop the PE out of its 2.4GHz p-state).
PV/rowsum consumption runs 2 s-blocks behind exp, crossing pair boundaries.
"""

import sys

sys.path.insert(0, "/opt/trn_rl_repo")

from collections import deque
from contextlib import ExitStack

import ml_dtypes
import numpy as np

import concourse.bass as bass  # noqa: F401
import concourse.tile as tile
from concourse import bacc, mybir
from concourse.bass_utils import run_bass_kernel_spmd

F32 = mybir.dt.float32
BF16 = mybir.dt.bfloat16
AF = mybir.ActivationFunctionType

D = 2048          # model dim
T = 2048          # tokens
DH = 128          # head dim
G = 4             # kv groups
HPG = 4           # query heads per group
QC = HPG * DH     # query cols per group = 512
ND = D // 128     # 16 contraction chunks
NTAU = 4          # t tiles of 512
TW = 512          # t tile width
SW = 2 * DH + QC  # strm0 row elems per d-chunk: wk|wv|wq = 768
SCALE = DH ** -0.5
NEG = -1e30

TRACE = False
TRACE_KW = {}
LAST_RESULTS = None

_CACHE = {}


def _body(ctx, tc, tens):
    nc = tc.nc
    (xtd, strmd, wod, bqd, bkd, bvd, maskd, identd, yb) = tens

    # PSUM: acc(2) + st(3) + ot(2) + rs(1) = 8 banks
    psacc = ctx.enter_context(tc.tile_pool(name="psacc", bufs=2, space="PSUM"))
    psst = ctx.enter_context(tc.tile_pool(name="psst", bufs=3, space="PSUM"))
    psot = ctx.enter_context(tc.tile_pool(name="psot", bufs=2, space="PSUM"))
    psrs = ctx.enter_context(tc.tile_pool(name="psrs", bufs=1, space="PSUM"))

    consts = ctx.enter_context(tc.tile_pool(name="consts", bufs=1))
    ptp = ctx.enter_context(tc.tile_pool(name="ptp", bufs=5))
    vts = ctx.enter_context(tc.tile_pool(name="vstage", bufs=2))
    ysp = ctx.enter_context(tc.tile_pool(name="ybounce", bufs=4))
    rsb = ctx.enter_context(tc.tile_pool(name="rsb", bufs=2))
    rbp = ctx.enter_context(tc.tile_pool(name="rcb", bufs=2))

    # persistent SBUF tiles
    xts = consts.tile([128, NTAU, ND, TW], BF16, tag="xts")
    strm = consts.tile([128, ND, SW], BF16, tag="strm")   # wk|wv|wq per d
    wot = consts.tile([128, HPG, D], BF16, tag="wot")
    bqt = consts.tile([128, HPG], F32, tag="bqt")
    bkt = consts.tile([128, 1], F32, tag="bkt")
    bvt = consts.tile([128, 1], F32, tag="bvt")
    maskt = consts.tile([128, 128], F32, tag="maskt")
    ident = consts.tile([128, 128], BF16, tag="ident")
    kt = consts.tile([128, T], BF16, tag="kt")
    qts = consts.tile([128, HPG, T], BF16, tag="qts")
    vv = consts.tile([128, ND, DH], BF16, tag="vv")
    ots = consts.tile([128, HPG, T], BF16, tag="ots")
    ones128 = consts.tile([128, 128], BF16, tag="ones128")
    rst = psrs.tile([128, TW], F32, tag="rs")

    def wk_d(d):
        return strm[:, d, 0:DH]

    def wv_d(d):
        return strm[:, d, DH:2 * DH]

    def wq_d(d, cb):
        return strm[:, d, 2 * DH + cb * 128:2 * DH + (cb + 1) * 128]

    # ---- DMA schedule ----
    # gpsimd SWDGE (bulk, aggregating): strm/x-tg0 interleaved per d-pair
    # in consumption order, then x tg1, tg2
    for p in range(8):
        nc.gpsimd.dma_start(strm[:, 2 * p:2 * p + 2, :],
                            strmd[:, 2 * p:2 * p + 2, :])
        nc.gpsimd.dma_start(xts[:, 0, 2 * p:2 * p + 2, :],
                            xtd[:, 0, 2 * p:2 * p + 2, :])
    for p in range(4):
        nc.gpsimd.dma_start(xts[:, 1, 4 * p:4 * p + 4, :],
                            xtd[:, 1, 4 * p:4 * p + 4, :])
    nc.gpsimd.dma_start(xts[:, 2], xtd[:, 2])
    # sync HWDGE: wo m0-7, x tg3 first half
    nc.sync.dma_start(wot[:, :, 0:1024], wod[:, :, 0:1024])
    nc.sync.dma_start(xts[:, 3, 0:8, :], xtd[:, 3, 0:8, :])
    # scalar HWDGE: small consts, wo m8-15, x tg3 second half
    nc.scalar.dma_start(bkt, bkd)
    nc.scalar.dma_start(bvt, bvd)
    nc.scalar.dma_start(bqt, bqd)
    nc.scalar.dma_start(ident, identd)
    nc.scalar.dma_start(maskt, maskd)
    nc.scalar.dma_start(wot[:, :, 1024:2048], wod[:, :, 1024:2048])
    nc.scalar.dma_start(xts[:, 3, 8:16, :], xtd[:, 3, 8:16, :])
    nc.vector.memset(ones128, 1.0)

    # ---- per-tau projection block: K/V/Q0-2 d-interleaved over 5
    # accumulators (consumes x/W chunks as they arrive), then Q3 ----
    def emit_block(tau):
        # Q3 joins the d-interleave only for tau0 (psot is fresh then; for
        # later blocks the psot ring slot would WAR against a normalize
        # still in flight, so Q3 runs as a short second pass instead).
        accK = psacc.tile([128, TW], F32, tag="acc", name="accK")
        accV = psacc.tile([128, TW], F32, tag="acc", name="accV")
        accQ = [psst.tile([128, TW], F32, tag="st", name=f"accQ{i}")
                for i in range(3)]
        accQ3 = psot.tile([128, TW], F32, tag="ot", name="accQ3") \
            if tau == 0 else None
        off = tau * TW
        for d in range(ND):
            xd = xts[:, tau, d, :]
            nc.tensor.matmul(accK, wk_d(d), xd, start=(d == 0),
                             stop=(d == ND - 1))
            nc.tensor.matmul(accV, wv_d(d), xd, start=(d == 0),
                             stop=(d == ND - 1))
            for cb in range(3):
                nc.tensor.matmul(accQ[cb], wq_d(d, cb), xd, start=(d == 0),
                                 stop=(d == ND - 1))
            if accQ3 is not None:
                nc.tensor.matmul(accQ3, wq_d(d, 3), xd, start=(d == 0),
                                 stop=(d == ND - 1))
        nc.vector.tensor_scalar_add(kt[:, off:off + TW], accK, bkt[:, 0:1])
        for cb in range(3):
            nc.vector.tensor_scalar_add(qts[:, cb, off:off + TW], accQ[cb],
                                        bqt[:, cb:cb + 1])
        if accQ3 is None:
            accQ3 = psacc.tile([128, TW], F32, tag="acc", name="accQ3")
            for d in range(ND):
                nc.tensor.matmul(accQ3, wq_d(d, 3), xts[:, tau, d, :],
                                 start=(d == 0), stop=(d == ND - 1))
        vtt = vts.tile([128, TW], BF16, tag="vt")
        nc.vector.tensor_scalar_add(vtt, accV, bvt[:, 0:1])
        for i in range(4):
            pv = psst.tile([128, 128], BF16, tag="st", name="pv")
            nc.tensor.transpose(pv, vtt[:, i * 128:(i + 1) * 128], ident)
            nc.vector.tensor_copy(vv[:, tau * 4 + i, :], pv)
        nc.vector.tensor_scalar_add(qts[:, 3, off:off + TW], accQ3,
                                    bqt[:, 3:4])

    # ---- output-projection block emitter ----
    def emit_oproj_block(tau, m):
        yp = psacc.tile([128, TW], F32, tag="acc", name="yp")
        for c in range(HPG):
            nc.tensor.matmul(yp, wot[:, c, m * 128:(m + 1) * 128],
                             ots[:, c, tau * TW:(tau + 1) * TW],
                             start=(c == 0), stop=(c == HPG - 1))
        ys = ysp.tile([128, TW], BF16, tag="ys")
        tail = tau == 3 or (tau == 2 and m >= 12)
        eng = nc.scalar if (tail and m % 2 == 1) else nc.vector
        if eng is nc.scalar:
            nc.scalar.copy(ys, yp)
        else:
            nc.vector.tensor_copy(ys, yp)
        dma_eng = nc.scalar if (tail and m % 2 == 1) else nc.sync
        dma_eng.dma_start(yb[m, tau], ys)

    # ---- attention ----
    # pending: PV + rowsum matmuls lag exp by 3 s-blocks. The rowsum uses a
    # ones [128,128] stationary so its PE tile config matches every other
    # matmul (an M=1 output costs a ~100ns array-reconfig on entry and exit);
    # all 128 result rows hold the rowsum, which doubles as the partition
    # broadcast the normalize needs.
    pending = deque()

    def consume_one():
        otp, pt, lo, sb, first, last, tau, j = pending.popleft()
        nc.tensor.matmul(otp[:, lo:], vv[:, sb, :], pt[:, lo:],
                         start=first, stop=last)
        nc.tensor.matmul(rst[:, lo:], ones128, pt[:, lo:],
                         start=first, stop=last)
        if last:
            # normalize chain, off the PE stream
            rss = rsb.tile([128, TW], F32, tag="rss")
            nc.vector.tensor_copy(rss, rst)
            rcb = rbp.tile([128, TW], F32, tag="rcb")
            nc.vector.reciprocal_approx_fast(rcb, rss)
            nc.vector.tensor_mul(ots[:, j, tau * TW:(tau + 1) * TW], otp, rcb)

    def emit_pair(tau, j):
        nsb = 4 * tau + 4
        otp = psot.tile([128, TW], F32, tag="ot")
        qoff = tau * TW
        for sb in range(nsb):
            di = sb - 4 * tau
            lo = di * 128 if di >= 0 else 0
            st = psst.tile([128, TW], F32, tag="st")
            nc.tensor.matmul(st[:, lo:], kt[:, sb * 128:(sb + 1) * 128],
                             qts[:, j, qoff + lo:qoff + TW],
                             start=True, stop=True)
            if di >= 0:
                nc.vector.tensor_add(st[:, lo:lo + 128], st[:, lo:lo + 128],
                                     maskt)
            pt = ptp.tile([128, TW], BF16, tag="pt")
            nc.scalar.activation(pt[:, lo:], st[:, lo:], AF.Exp, scale=SCALE)
            pending.append((otp, pt, lo, sb, sb == 0, sb == nsb - 1, tau, j))
            while len(pending) > 4:
                consume_one()

    # ---- main schedule ----
    emit_block(0)
    for tau in range(NTAU):
        for j in range(HPG):
            emit_pair(tau, j)
            if tau >= 1 and not (tau == 3 and j == 3):
                for m in range(4 * j, 4 * j + 4):
                    emit_oproj_block(tau - 1, m)
        if tau < 3:
            emit_block(tau + 1)

    # tail: remaining oproj(2) blocks interleaved with the pending flush
    emit_oproj_block(2, 12)
    if pending:
        consume_one()
    emit_oproj_block(2, 13)
    if pending:
        consume_one()
    emit_oproj_block(2, 14)
    while pending:
        consume_one()
    emit_oproj_block(2, 15)
    for m in range(ND):
        emit_oproj_block(3, m)


def _build_nc():
    if "nc" in _CACHE:
        return _CACHE["nc"]
    nc = bacc.Bacc("TRN2", target_bir_lowering=False, debug=False)
    xtd = nc.dram_tensor("xt", [128, NTAU, ND, TW], BF16,
                         kind="ExternalInput").ap()
    strmd = nc.dram_tensor("strm", [128, ND, SW], BF16,
                           kind="ExternalInput").ap()
    wod = nc.dram_tensor("wo", [128, HPG, D], BF16, kind="ExternalInput").ap()
    bqd = nc.dram_tensor("bq", [128, HPG], F32, kind="ExternalInput").ap()
    bkd = nc.dram_tensor("bk", [128, 1], F32, kind="ExternalInput").ap()
    bvd = nc.dram_tensor("bv", [128, 1], F32, kind="ExternalInput").ap()
    maskd = nc.dram_tensor("mask", [128, 128], F32, kind="ExternalInput").ap()
    identd = nc.dram_tensor("ident", [128, 128], BF16,
                            kind="ExternalInput").ap()
    yb = nc.dram_tensor("yb", [ND, NTAU, 128, TW], BF16,
                        kind="ExternalOutput").ap()

    tens = (xtd, strmd, wod, bqd, bkd, bvd, maskd, identd, yb)
    with tile.TileContext(nc) as tc, ExitStack() as ctx:
        _body(ctx, tc, tens)
    nc.compile()
    _CACHE["nc"] = nc
    return nc


def _host_consts():
    p = np.arange(128)[:, None]
    f = np.arange(128)[None, :]
    masks = np.where(f >= p, 0.0, NEG).astype(np.float32)
    ident = np.eye(128, dtype=ml_dtypes.bfloat16)
    return masks, ident


def kernel(x, Wq, bq, Wk, bk, Wv, bv, Wo, bo):
    global LAST_RESULTS
    x = np.asarray(x, np.float32)
    Wq = np.asarray(Wq, np.float32)
    Wk = np.asarray(Wk, np.float32)
    Wv = np.asarray(Wv, np.float32)
    Wo = np.asarray(Wo, np.float32)
    bq = np.asarray(bq, np.float32)
    bk = np.asarray(bk, np.float32)
    bv = np.asarray(bv, np.float32)
    bo = np.asarray(bo, np.float32)

    nc = _build_nc()
    masks, ident = _host_consts()
    bf = lambda a: np.ascontiguousarray(a).astype(ml_dtypes.bfloat16)

    in_maps = []
    for c in range(8):
        b, g = divmod(c, G)
        xt = x[b].T.reshape(ND, 128, NTAU, TW).transpose(1, 2, 0, 3)
        wk = Wk[:, g * DH:(g + 1) * DH].reshape(ND, 128, DH).transpose(1, 0, 2)
        wv = Wv[:, g * DH:(g + 1) * DH].reshape(ND, 128, DH).transpose(1, 0, 2)
        wq = Wq[:, g * QC:(g + 1) * QC].reshape(ND, 128, QC).transpose(1, 0, 2)
        strm = np.concatenate([wk, wv, wq], axis=2)
        wo = Wo[g * QC:(g + 1) * QC, :].reshape(HPG, 128, D).transpose(1, 0, 2)
        in_maps.append({
            "xt": bf(xt),
            "strm": bf(strm),
            "wo": bf(wo),
            "bq": np.ascontiguousarray(
                bq[g * QC:(g + 1) * QC].reshape(HPG, 128).T),
            "bk": np.ascontiguousarray(
                bk[g * DH:(g + 1) * DH].reshape(128, 1)),
            "bv": np.ascontiguousarray(
                bv[g * DH:(g + 1) * DH].reshape(128, 1)),
            "mask": masks,
            "ident": ident,
        })

    res = run_bass_kernel_spmd(nc, in_maps, list(range(8)), trace=TRACE,
                               **TRACE_KW)
    LAST_RESULTS = res

    y = np.empty((2, T, D), np.float32)
    for b in range(2):
        acc = np.zeros((D, T), np.float32)
        for g in range(G):
            blk = res.results[b * G + g]["yb"].astype(np.float32)
            acc += blk.transpose(0, 2, 1, 3).reshape(D, T)
        y[b] = acc.T + bo
    return y
